# revision 58
# baseline (speedup 1.0000x reference)
"""AMPNNConv distributed Trainium2 kernel.

Math (reformulated from the reference, numerically equivalent):
    w_m = efeat @ W_msg + b_msg          [E, 16*16]
    w_a = efeat @ W_attn + b_attn        [E, 16*16]
    h   = feat[src]                      [E, 16]   (broadcast over out dim)
    ex  = exp(w_a * h)                   (no max-subtraction: |w_a*h| <~ 8,
                                          exp is f32-safe; softmax is
                                          shift-invariant)
    num[n] = sum_{e: dst[e]=n} (w_m*h) * ex
    den[n] = sum_{e: dst[e]=n} ex
    out[n, j] = sum_i num[n,i,j] / den[n,i,j]

Sharding: edges sorted by dst on host; core c owns nodes
[c*3750, (c+1)*3750) so every segment-sum is core-local (no collectives).
Within a core, edges are bucketed into 30 windows of 128 destination
nodes; each 128-edge tile targets one window and the segment-sum is a
one-hot matmul accumulated in PSUM.

Key structure (v8): the per-edge h-broadcast multiply is linearized into
the TensorEngine by sending G[e,(k,i)] = efeat[e,k]*h[e,i] from the host:
    (w*h)[e,(i,j)] = sum_k G[e,(k,i)] * W[k,(i,j)]
With K=(k,i) split into two i-halves of 128 rows each, two K=128 bf16
matmuls per tile produce both h-scaled weight tensors (msg|attn) straight
into PSUM.  Lanes are ordered (i-block, msg/attn, i%8, j); the final
sum over i is permutation-invariant so only j-order matters.

Performance notes (measured on TRN2):
- bf16 matmuls at K=128 stream ~216ns/512 cols; K<128 runs 2x slower,
  f32 runs LOW_HIGH multi-pass.  LDWEIGHTS overlaps in-flight matmuls.
- ACT applies exp reading PSUM directly; DVE's only heavy op is
  p1 = e1*ex (PSUM read); GPSIMD only issues DMA.
- num/den stash to SBUF; division (exp(-ln(d))) runs in 4 window
  batches overlapped with the main loop.
"""

import numpy as np

N_NODES = 30000
N_EDGES = 300000
F = 16              # in = out = edge_dim
C = 8               # cores
NPC = N_NODES // C  # nodes per core = 3750
P = 128
W = (NPC + P - 1) // P  # windows per core = 30
NPAD = W * P            # padded nodes per core = 3840


def _prep(feat, efeat, W_msg, b_msg, W_attn, b_attn, src, dst):
    """Host-side shard/sort/pad. Returns (in_maps, Tw, T, has_bias)."""
    import ml_dtypes
    f32 = np.float32
    bf16 = ml_dtypes.bfloat16
    order = np.argsort(dst, kind="stable")
    dsts = dst[order].astype(np.int64)
    core_of = dsts // NPC
    nloc = dsts - core_of * NPC
    win = nloc // P
    wloc = nloc % P

    # tiles per window: max over cores, >= 1
    cnt = np.zeros((C, W), np.int64)
    np.add.at(cnt, (core_of, win), 1)
    Tw = np.maximum(1, -(-cnt.max(axis=0) // P)).astype(np.int64)
    if Tw.sum() % 2:
        Tw[-1] += 1          # keep T even (ops pair tiles)
    T = int(Tw.sum())
    wb = np.concatenate([[0], np.cumsum(Tw)])[:-1]  # window tile base

    has_bias = bool(np.any(b_msg) or np.any(b_attn))

    # W-tilde constants: wt[blk][(k,i8), (a,i8',j)] = 1{i8==i8'} *
    # W_a[k, (blk*8+i8)*16+j], lane layout (a, i8, j) within each block
    Wab = [W_msg.reshape(F, F, F), W_attn.reshape(F, F, F)]
    wt = np.zeros((2, P, 2 * P), f32)
    for blk in range(2):
        for k in range(F):
            for i8 in range(8):
                for a in range(2):
                    wt[blk, k * 8 + i8,
                       a * 128 + i8 * 16:a * 128 + i8 * 16 + 16] = \
                        Wab[a][k, blk * 8 + i8]
    # bias matmul constant: bt[i, (blk,a,i8,j)] = 1{i==blk*8+i8}*b_a[i,j]
    bt = np.zeros((F, 4 * P), f32)
    if has_bias:
        bb = [b_msg.reshape(F, F), b_attn.reshape(F, F)]
        for i in range(F):
            blk, i8 = divmod(i, 8)
            for a in range(2):
                c0 = blk * 256 + a * 128 + i8 * 16
                bt[i, c0:c0 + 16] = bb[a][i]

    in_maps = []
    for c in range(C):
        m = core_of == c
        e_idx = order[m]          # original edge ids, sorted by local node
        w_c = win[m]
        wl = wloc[m]
        cc = cnt[c]
        run_starts = np.concatenate([[0], np.cumsum(cc)])[:-1]
        rank = np.arange(m.sum()) - np.repeat(run_starts, cc)
        slot = (wb[w_c] * P + rank).astype(np.int64)

        ef_e = efeat[e_idx]                 # [m, 16]
        h_e = feat[src[e_idx]]              # [m, 16]
        G = ef_e[:, :, None] * h_e[:, None, :]   # [m, k, i]
        ga = np.zeros((P, T * P), bf16)
        gb = np.zeros((P, T * P), bf16)
        ga[:, slot] = G[:, :, :8].reshape(-1, P).T
        gb[:, slot] = G[:, :, 8:].reshape(-1, P).T
        oh = np.zeros((T * P, P), bf16)
        oh[slot, wl] = 1.0
        oh_t = np.ascontiguousarray(oh.reshape(T, P, P).transpose(1, 0, 2))
        im = {"ga": ga, "gb": gb, "oh": oh_t,
              "wta": wt[0].astype(bf16), "wtb": wt[1].astype(bf16)}
        if has_bias:
            hT = np.zeros((F, T * P), bf16)
            hT[:, slot] = h_e.T
            im["hT"] = hT
            im["bt"] = bt.astype(bf16)
        in_maps.append(im)
    return in_maps, Tw, T, has_bias


def _build(Tw, T, has_bias):
    import concourse.bass as bass
    import concourse.mybir as mybir
    from concourse import bacc, tile

    f32 = mybir.dt.float32
    bf16 = mybir.dt.bfloat16
    mult = mybir.AluOpType.mult

    nc = bacc.Bacc(None, target_bir_lowering=False)
    ga_ext = nc.declare_dram_parameter("ga", [P, T * P], bf16,
                                       isOutput=False)
    gb_ext = nc.declare_dram_parameter("gb", [P, T * P], bf16,
                                       isOutput=False)
    oh_ext = nc.declare_dram_parameter("oh", [P, T, P], bf16,
                                       isOutput=False)
    wta_ext = nc.declare_dram_parameter("wta", [P, 2 * P], bf16,
                                        isOutput=False)
    wtb_ext = nc.declare_dram_parameter("wtb", [P, 2 * P], bf16,
                                        isOutput=False)
    if has_bias:
        hT_ext = nc.declare_dram_parameter("hT", [F, T * P], bf16,
                                           isOutput=False)
        bt_ext = nc.declare_dram_parameter("bt", [F, 4 * P], bf16,
                                           isOutput=False)
    out_ext = nc.declare_dram_parameter("out", [NPAD, F], f32, isOutput=True)

    CH = 8  # tiles per DMA chunk (must be even)

    with tile.TileContext(nc) as tc:
        with (
            tc.tile_pool(name="const", bufs=1) as constp,
            tc.tile_pool(name="chunk", bufs=4) as chunkp,
            tc.tile_pool(name="work", bufs=4) as workp,
            tc.tile_pool(name="stash", bufs=1) as stashp,
            tc.tile_pool(name="tail", bufs=1) as tailp,
            tc.tile_pool(name="wps", bufs=3, space=bass.MemorySpace.PSUM) as wpsp,
            tc.tile_pool(name="acc", bufs=2, space=bass.MemorySpace.PSUM) as accp,
        ):
            wta = constp.tile([P, 2 * P], bf16, tag="wta")
            nc.sync.dma_start(wta[:], wta_ext[:])
            wtb = constp.tile([P, 2 * P], bf16, tag="wtb")
            nc.sync.dma_start(wtb[:], wtb_ext[:])
            if has_bias:
                btc = constp.tile([F, 4 * P], bf16, tag="btc")
                nc.sync.dma_start(btc[:], bt_ext[:])
            eps = constp.tile([P, 1], f32, tag="eps")
            nc.vector.memset(eps[:], 1e-30)

            # per-window num/den stash in SBUF (f32), acc layout
            # preserved: [w, (blk, a, c)] with a=0 num, a=1 den
            nd = stashp.tile([P, W, 4 * P], f32, tag="nd")
            ndv = nd[:].rearrange("p w (blk a c) -> p w blk a c",
                                  blk=2, a=2)

            n_chunks = (T + CH - 1) // CH
            ga_ch = [None] * n_chunks
            gb_ch = [None] * n_chunks
            oh_ch = [None] * n_chunks
            hT_ch = [None] * n_chunks

            def load_chunk(ci):
                t0 = ci * CH
                n = min(CH, T - t0)
                ga = chunkp.tile([P, CH * P], bf16, tag="gach")
                nc.sync.dma_start(ga[:, :n * P],
                                  ga_ext[:, t0 * P:(t0 + n) * P])
                gb = chunkp.tile([P, CH * P], bf16, tag="gbch")
                nc.gpsimd.dma_start(gb[:, :n * P],
                                    gb_ext[:, t0 * P:(t0 + n) * P])
                ohh = chunkp.tile([P, CH, P], bf16, tag="ohch")
                nc.gpsimd.dma_start(ohh[:, :n, :], oh_ext[:, t0:t0 + n, :])
                ga_ch[ci], gb_ch[ci], oh_ch[ci] = ga, gb, ohh
                if has_bias:
                    ht = chunkp.tile([F, CH * P], bf16, tag="htch")
                    nc.sync.dma_start(ht[:, :n * P],
                                      hT_ext[:, t0 * P:(t0 + n) * P])
                    hT_ch[ci] = ht

            # flat tile order; windows are contiguous runs of tiles
            tile_win = np.repeat(np.arange(W), Tw)
            win_last = np.concatenate([[0], np.cumsum(Tw)])[1:] - 1
            win_first = np.concatenate([[0], np.cumsum(Tw)])[:-1]

            # division tails run in batches so they overlap the main
            # loop; the last batch is tiny so the serial tail is short
            NBATCH = 4
            bounds = [W * (b + 1) // NBATCH for b in range(NBATCH)]

            def emit_tail(w0, w1):
                nw = w1 - w0
                WB = W // NBATCH + 1
                rden = tailp.tile([P, WB, 2, P], f32, tag="rden")
                nc.scalar.activation(rden[:, :nw], ndv[:, w0:w1, :, 1, :],
                                     mybir.ActivationFunctionType.Ln,
                                     bias=eps[:])
                nc.scalar.activation(rden[:, :nw], rden[:, :nw],
                                     mybir.ActivationFunctionType.Exp,
                                     scale=-1.0)
                # ft compacts the (blk, c) lanes to contiguous [2*P]
                ft = tailp.tile([P, WB, 2, P], f32, tag="ft")
                nc.vector.tensor_tensor(ft[:, :nw], ndv[:, w0:w1, :, 0, :],
                                        rden[:, :nw], mult)
                outw = tailp.tile([P, WB, F], f32, tag="outw")
                ft_flat = ft[:].rearrange("p w blk c -> p w (blk c)")
                nc.vector.tensor_reduce(
                    outw[:, :nw, :],
                    ft_flat[:, :nw, :].rearrange("p w (i j) -> p w j i",
                                                 i=F),
                    mybir.AxisListType.X, mybir.AluOpType.add)
                nc.sync.dma_start(
                    out_ext[w0 * P:w1 * P, :].rearrange(
                        "(w p) f -> p w f", p=P),
                    outw[:, :nw, :])

            acc_of_win = {}
            done_w = 0
            for tp in range(T // 2):
                t0 = 2 * tp
                ci, si = divmod(t0, CH)
                if ga_ch[ci] is None:
                    load_chunk(ci)

                wps = wpsp.tile([P, 8 * P], f32, tag="wps")
                pay = workp.tile([P, 8 * P], bf16, tag="pay")
                for pi in range(2):
                    sl = slice((si + pi) * P, (si + pi + 1) * P)
                    nc.tensor.matmul(wps[:, pi * 512:pi * 512 + 256],
                                     ga_ch[ci][:, sl], wta[:],
                                     start=True, stop=not has_bias,
                                     skip_group_check=True)
                    if has_bias:
                        nc.tensor.matmul(
                            wps[:, pi * 512:pi * 512 + 256],
                            hT_ch[ci][:, sl], btc[:, :256],
                            start=False, stop=True,
                            skip_group_check=True)
                    nc.tensor.matmul(wps[:, pi * 512 + 256:pi * 512 + 512],
                                     gb_ch[ci][:, sl], wtb[:],
                                     start=True, stop=not has_bias,
                                     skip_group_check=True)
                    if has_bias:
                        nc.tensor.matmul(
                            wps[:, pi * 512 + 256:pi * 512 + 512],
                            hT_ch[ci][:, sl], btc[:, 256:],
                            start=False, stop=True,
                            skip_group_check=True)
                # lanes per tile: (blk, a, i8, j); attn = a=1 halves.
                # exp/p1 run per tile so the scatter chain starts sooner
                wv = wps[:].rearrange("p (t blk a c) -> p t blk a c",
                                      t=2, blk=2, a=2)
                pv = pay[:].rearrange("p (t blk a c) -> p t blk a c",
                                      t=2, blk=2, a=2)
                for pi in range(2):
                    nc.scalar.activation(pv[:, pi, :, 1, :],
                                         wv[:, pi, :, 1, :],
                                         mybir.ActivationFunctionType.Exp)
                    nc.vector.tensor_tensor(pv[:, pi, :, 0, :],
                                            wv[:, pi, :, 0, :],
                                            pv[:, pi, :, 1, :], mult)

                for pi in range(2):
                    t = t0 + pi
                    w = int(tile_win[t])
                    if w not in acc_of_win:
                        acc = accp.tile([P, 4 * P], f32, tag="acc")
                        acc_of_win[w] = acc
                    acc = acc_of_win[w]
                    nc.tensor.matmul(acc[:],
                                     oh_ch[ci][:, si + pi, :],
                                     pay[:, pi * 512:(pi + 1) * 512],
                                     start=(t == win_first[w]),
                                     stop=(t == win_last[w]),
                                     skip_group_check=True)
                    if t == win_last[w]:
                        # stash whole acc (one copy); division batched
                        nc.scalar.copy(nd[:, w, :], acc[:])
                        del acc_of_win[w]
                        if w + 1 in bounds:
                            emit_tail(done_w, w + 1)
                            done_w = w + 1
                if si + 1 == CH - 1:
                    ga_ch[ci] = gb_ch[ci] = oh_ch[ci] = hT_ch[ci] = None
    nc.compile()
    return nc


TRACE = False          # set True (e.g. from test.py) to capture a profile
TRACE_DIR = None       # where to keep NTFF/perfetto artifacts
LAST_RESULT = None     # BassKernelResults of the last run (for profiling)


def kernel(feat, efeat, W_msg, b_msg, W_attn, b_attn, src, dst):
    global LAST_RESULT
    from concourse.bass_utils import run_bass_kernel_spmd

    in_maps, Tw, T, has_bias = _prep(feat, efeat, W_msg, b_msg, W_attn,
                                     b_attn, src, dst)
    nc = _build(Tw, T, has_bias)
    res = run_bass_kernel_spmd(nc, in_maps, core_ids=list(range(C)),
                               trace=TRACE, tmpdir=TRACE_DIR)
    LAST_RESULT = res
    out = np.empty((N_NODES, F), np.float32)
    for c in range(C):
        out[c * NPC:(c + 1) * NPC] = res.results[c]["out"][:NPC]
    return out


# revision 59
# speedup vs baseline: 1.1977x; 1.1977x over previous
"""AMPNNConv distributed Trainium2 kernel.

Math (reformulated from the reference, numerically equivalent):
    w_m = efeat @ W_msg + b_msg          [E, 16*16]
    w_a = efeat @ W_attn + b_attn        [E, 16*16]
    h   = feat[src]                      [E, 16]   (broadcast over out dim)
    ex  = exp(w_a * h)                   (no max-subtraction: |w_a*h| <~ 8,
                                          exp is f32-safe; softmax is
                                          shift-invariant)
    num[n] = sum_{e: dst[e]=n} (w_m*h) * ex
    den[n] = sum_{e: dst[e]=n} ex
    out[n, j] = sum_i num[n,i,j] / den[n,i,j]

Sharding: edges sorted by dst on host; core c owns nodes
[c*3750, (c+1)*3750) so every segment-sum is core-local (no collectives).
Within a core, edges are bucketed into 30 windows of 128 destination
nodes; each 128-edge tile targets one window and the segment-sum is a
one-hot matmul accumulated in PSUM.

Key structure (v8): the per-edge h-broadcast multiply is linearized into
the TensorEngine by sending G[e,(k,i)] = efeat[e,k]*h[e,i] from the host:
    (w*h)[e,(i,j)] = sum_k G[e,(k,i)] * W[k,(i,j)]
With K=(k,i) split into two i-halves of 128 rows each, two K=128 bf16
matmuls per tile produce both h-scaled weight tensors (msg|attn) straight
into PSUM.  Lanes are ordered (i-block, msg/attn, i%8, j); the final
sum over i is permutation-invariant so only j-order matters.

Performance notes (measured on TRN2):
- bf16 matmuls at K=128 stream ~216ns/512 cols; K<128 runs 2x slower,
  f32 runs LOW_HIGH multi-pass.  LDWEIGHTS overlaps in-flight matmuls.
- ACT applies exp reading PSUM directly; DVE's only heavy op is
  p1 = e1*ex (PSUM read); GPSIMD only issues DMA.
- num/den stash to SBUF; division (exp(-ln(d))) runs in 4 window
  batches overlapped with the main loop.
"""

import numpy as np

N_NODES = 30000
N_EDGES = 300000
F = 16              # in = out = edge_dim
C = 8               # cores
NPC = N_NODES // C  # nodes per core = 3750
P = 128
W = (NPC + P - 1) // P  # windows per core = 30
NPAD = W * P            # padded nodes per core = 3840


def _prep(feat, efeat, W_msg, b_msg, W_attn, b_attn, src, dst):
    """Host-side shard/sort/pad. Returns (in_maps, Tw, T, has_bias)."""
    import ml_dtypes
    f32 = np.float32
    bf16 = ml_dtypes.bfloat16
    order = np.argsort(dst, kind="stable")
    dsts = dst[order].astype(np.int64)
    core_of = dsts // NPC
    nloc = dsts - core_of * NPC
    win = nloc // P
    wloc = nloc % P

    # tiles per window: max over cores, >= 1
    cnt = np.zeros((C, W), np.int64)
    np.add.at(cnt, (core_of, win), 1)
    Tw = np.maximum(1, -(-cnt.max(axis=0) // P)).astype(np.int64)
    if Tw.sum() % 2:
        Tw[-1] += 1          # keep T even (ops pair tiles)
    T = int(Tw.sum())
    wb = np.concatenate([[0], np.cumsum(Tw)])[:-1]  # window tile base

    has_bias = bool(np.any(b_msg) or np.any(b_attn))

    # W-tilde constants: wt[blk][(k,i8), (a,i8',j)] = 1{i8==i8'} *
    # W_a[k, (blk*8+i8)*16+j], lane layout (a, i8, j) within each block
    Wab = [W_msg.reshape(F, F, F), W_attn.reshape(F, F, F)]
    wt = np.zeros((2, P, 2 * P), f32)
    for blk in range(2):
        for k in range(F):
            for i8 in range(8):
                for a in range(2):
                    wt[blk, k * 8 + i8,
                       a * 128 + i8 * 16:a * 128 + i8 * 16 + 16] = \
                        Wab[a][k, blk * 8 + i8]
    # bias matmul constant: bt[i, (blk,a,i8,j)] = 1{i==blk*8+i8}*b_a[i,j]
    bt = np.zeros((F, 4 * P), f32)
    if has_bias:
        bb = [b_msg.reshape(F, F), b_attn.reshape(F, F)]
        for i in range(F):
            blk, i8 = divmod(i, 8)
            for a in range(2):
                c0 = blk * 256 + a * 128 + i8 * 16
                bt[i, c0:c0 + 16] = bb[a][i]

    in_maps = []
    for c in range(C):
        m = core_of == c
        e_idx = order[m]          # original edge ids, sorted by local node
        w_c = win[m]
        wl = wloc[m]
        cc = cnt[c]
        run_starts = np.concatenate([[0], np.cumsum(cc)])[:-1]
        rank = np.arange(m.sum()) - np.repeat(run_starts, cc)
        slot = (wb[w_c] * P + rank).astype(np.int64)

        ef_e = efeat[e_idx]                 # [m, 16]
        h_e = feat[src[e_idx]]              # [m, 16]
        G = ef_e[:, :, None] * h_e[:, None, :]   # [m, k, i]
        ga = np.zeros((P, T * P), bf16)
        gb = np.zeros((P, T * P), bf16)
        ga[:, slot] = G[:, :, :8].reshape(-1, P).T
        gb[:, slot] = G[:, :, 8:].reshape(-1, P).T
        oh = np.zeros((T * P, P), bf16)
        oh[slot, wl] = 1.0
        oh_t = np.ascontiguousarray(oh.reshape(T, P, P).transpose(1, 0, 2))
        im = {"ga": ga, "gb": gb, "oh": oh_t,
              "wta": wt[0].astype(bf16), "wtb": wt[1].astype(bf16)}
        if has_bias:
            hT = np.zeros((F, T * P), bf16)
            hT[:, slot] = h_e.T
            im["hT"] = hT
            im["bt"] = bt.astype(bf16)
        in_maps.append(im)
    return in_maps, Tw, T, has_bias


def _build(Tw, T, has_bias):
    import concourse.bass as bass
    import concourse.mybir as mybir
    from concourse import bacc, tile

    f32 = mybir.dt.float32
    bf16 = mybir.dt.bfloat16
    mult = mybir.AluOpType.mult

    nc = bacc.Bacc(None, target_bir_lowering=False)
    ga_ext = nc.declare_dram_parameter("ga", [P, T * P], bf16,
                                       isOutput=False)
    gb_ext = nc.declare_dram_parameter("gb", [P, T * P], bf16,
                                       isOutput=False)
    oh_ext = nc.declare_dram_parameter("oh", [P, T, P], bf16,
                                       isOutput=False)
    wta_ext = nc.declare_dram_parameter("wta", [P, 2 * P], bf16,
                                        isOutput=False)
    wtb_ext = nc.declare_dram_parameter("wtb", [P, 2 * P], bf16,
                                        isOutput=False)
    if has_bias:
        hT_ext = nc.declare_dram_parameter("hT", [F, T * P], bf16,
                                           isOutput=False)
        bt_ext = nc.declare_dram_parameter("bt", [F, 4 * P], bf16,
                                           isOutput=False)
    out_ext = nc.declare_dram_parameter("out", [NPAD, F], f32, isOutput=True)

    CH = 8  # tiles per DMA chunk (must be even)

    with tile.TileContext(nc) as tc:
        with (
            tc.tile_pool(name="const", bufs=1) as constp,
            tc.tile_pool(name="chunk", bufs=4) as chunkp,
            tc.tile_pool(name="work", bufs=4) as workp,
            tc.tile_pool(name="stash", bufs=1) as stashp,
            tc.tile_pool(name="tail", bufs=1) as tailp,
            tc.tile_pool(name="wps", bufs=3, space=bass.MemorySpace.PSUM) as wpsp,
            tc.tile_pool(name="acc", bufs=2, space=bass.MemorySpace.PSUM) as accp,
        ):
            wta = constp.tile([P, 2 * P], bf16, tag="wta")
            nc.sync.dma_start(wta[:], wta_ext[:])
            wtb = constp.tile([P, 2 * P], bf16, tag="wtb")
            nc.sync.dma_start(wtb[:], wtb_ext[:])
            if has_bias:
                btc = constp.tile([F, 4 * P], bf16, tag="btc")
                nc.sync.dma_start(btc[:], bt_ext[:])
            eps = constp.tile([P, 1], f32, tag="eps")
            nc.vector.memset(eps[:], 1e-30)

            # per-window num/den stash in SBUF (f32), acc layout
            # preserved: [w, (blk, a, c)] with a=0 num, a=1 den
            nd = stashp.tile([P, W, 4 * P], f32, tag="nd")
            ndv = nd[:].rearrange("p w (blk a c) -> p w blk a c",
                                  blk=2, a=2)

            n_chunks = (T + CH - 1) // CH
            ga_ch = [None] * n_chunks
            gb_ch = [None] * n_chunks
            oh_ch = [None] * n_chunks
            hT_ch = [None] * n_chunks

            def load_chunk(ci):
                t0 = ci * CH
                n = min(CH, T - t0)
                ga = chunkp.tile([P, CH * P], bf16, tag="gach")
                nc.sync.dma_start(ga[:, :n * P],
                                  ga_ext[:, t0 * P:(t0 + n) * P])
                gb = chunkp.tile([P, CH * P], bf16, tag="gbch")
                nc.gpsimd.dma_start(gb[:, :n * P],
                                    gb_ext[:, t0 * P:(t0 + n) * P])
                ohh = chunkp.tile([P, CH, P], bf16, tag="ohch")
                nc.gpsimd.dma_start(ohh[:, :n, :], oh_ext[:, t0:t0 + n, :])
                ga_ch[ci], gb_ch[ci], oh_ch[ci] = ga, gb, ohh
                if has_bias:
                    ht = chunkp.tile([F, CH * P], bf16, tag="htch")
                    nc.sync.dma_start(ht[:, :n * P],
                                      hT_ext[:, t0 * P:(t0 + n) * P])
                    hT_ch[ci] = ht

            # flat tile order; windows are contiguous runs of tiles
            tile_win = np.repeat(np.arange(W), Tw)
            win_last = np.concatenate([[0], np.cumsum(Tw)])[1:] - 1
            win_first = np.concatenate([[0], np.cumsum(Tw)])[:-1]

            # division tails run in batches so they overlap the main
            # loop; the last batch is tiny so the serial tail is short
            NBATCH = 4
            bounds = [W * (b + 1) // NBATCH for b in range(NBATCH)]

            def emit_tail(w0, w1):
                nw = w1 - w0
                WB = W // NBATCH + 1
                rden = tailp.tile([P, WB, 2, P], f32, tag="rden")
                nc.scalar.activation(rden[:, :nw], ndv[:, w0:w1, :, 1, :],
                                     mybir.ActivationFunctionType.Ln,
                                     bias=eps[:])
                nc.scalar.activation(rden[:, :nw], rden[:, :nw],
                                     mybir.ActivationFunctionType.Exp,
                                     scale=-1.0)
                # ft compacts the (blk, c) lanes to contiguous [2*P]
                ft = tailp.tile([P, WB, 2, P], f32, tag="ft")
                nc.vector.tensor_tensor(ft[:, :nw], ndv[:, w0:w1, :, 0, :],
                                        rden[:, :nw], mult)
                outw = tailp.tile([P, WB, F], f32, tag="outw")
                ft_flat = ft[:].rearrange("p w blk c -> p w (blk c)")
                nc.vector.tensor_reduce(
                    outw[:, :nw, :],
                    ft_flat[:, :nw, :].rearrange("p w (i j) -> p w j i",
                                                 i=F),
                    mybir.AxisListType.X, mybir.AluOpType.add)
                nc.sync.dma_start(
                    out_ext[w0 * P:w1 * P, :].rearrange(
                        "(w p) f -> p w f", p=P),
                    outw[:, :nw, :])

            acc_of_win = {}
            done_w = 0
            for tp in range(T // 2):
                t0 = 2 * tp
                ci, si = divmod(t0, CH)
                if ga_ch[ci] is None:
                    load_chunk(ci)

                wps = wpsp.tile([P, 8 * P], f32, tag="wps")
                pay = workp.tile([P, 8 * P], bf16, tag="pay")
                for pi in range(2):
                    sl = slice((si + pi) * P, (si + pi + 1) * P)
                    nc.tensor.matmul(wps[:, pi * 512:pi * 512 + 256],
                                     ga_ch[ci][:, sl], wta[:],
                                     start=True, stop=not has_bias,
                                     skip_group_check=True)
                    if has_bias:
                        nc.tensor.matmul(
                            wps[:, pi * 512:pi * 512 + 256],
                            hT_ch[ci][:, sl], btc[:, :256],
                            start=False, stop=True,
                            skip_group_check=True)
                    nc.tensor.matmul(wps[:, pi * 512 + 256:pi * 512 + 512],
                                     gb_ch[ci][:, sl], wtb[:],
                                     start=True, stop=not has_bias,
                                     skip_group_check=True)
                    if has_bias:
                        nc.tensor.matmul(
                            wps[:, pi * 512 + 256:pi * 512 + 512],
                            hT_ch[ci][:, sl], btc[:, 256:],
                            start=False, stop=True,
                            skip_group_check=True)
                # lanes per tile: (blk, a, i8, j); attn = a=1 halves
                wv = wps[:].rearrange("p (t blk a c) -> p t blk a c",
                                      t=2, blk=2, a=2)
                pv = pay[:].rearrange("p (t blk a c) -> p t blk a c",
                                      t=2, blk=2, a=2)
                nc.scalar.activation(pv[:, :, :, 1, :], wv[:, :, :, 1, :],
                                     mybir.ActivationFunctionType.Exp)
                nc.vector.tensor_tensor(pv[:, :, :, 0, :],
                                        wv[:, :, :, 0, :],
                                        pv[:, :, :, 1, :], mult)

                for pi in range(2):
                    t = t0 + pi
                    w = int(tile_win[t])
                    if w not in acc_of_win:
                        acc = accp.tile([P, 4 * P], f32, tag="acc")
                        acc_of_win[w] = acc
                    acc = acc_of_win[w]
                    nc.tensor.matmul(acc[:],
                                     oh_ch[ci][:, si + pi, :],
                                     pay[:, pi * 512:(pi + 1) * 512],
                                     start=(t == win_first[w]),
                                     stop=(t == win_last[w]),
                                     skip_group_check=True)
                    if t == win_last[w]:
                        # stash whole acc (one copy); division batched
                        nc.scalar.copy(nd[:, w, :], acc[:])
                        del acc_of_win[w]
                        if w + 1 in bounds:
                            emit_tail(done_w, w + 1)
                            done_w = w + 1
                if si + 1 == CH - 1:
                    ga_ch[ci] = gb_ch[ci] = oh_ch[ci] = hT_ch[ci] = None
    nc.compile()
    return nc


TRACE = False          # set True (e.g. from test.py) to capture a profile
TRACE_DIR = None       # where to keep NTFF/perfetto artifacts
LAST_RESULT = None     # BassKernelResults of the last run (for profiling)


def kernel(feat, efeat, W_msg, b_msg, W_attn, b_attn, src, dst):
    global LAST_RESULT
    from concourse.bass_utils import run_bass_kernel_spmd

    in_maps, Tw, T, has_bias = _prep(feat, efeat, W_msg, b_msg, W_attn,
                                     b_attn, src, dst)
    nc = _build(Tw, T, has_bias)
    res = run_bass_kernel_spmd(nc, in_maps, core_ids=list(range(C)),
                               trace=TRACE, tmpdir=TRACE_DIR)
    LAST_RESULT = res
    out = np.empty((N_NODES, F), np.float32)
    for c in range(C):
        out[c * NPC:(c + 1) * NPC] = res.results[c]["out"][:NPC]
    return out
